# revision 2
# baseline (speedup 1.0000x reference)
"""KitNET anomaly-detection ensemble (25 tiny tied-weight autoencoders) on 8 Trainium2 cores.

v2 strategy (vs v1's dense gather-folded matmuls):
  - Data-parallel over batch: each core processes BC = B/8 = 16384 samples.
  - The host hands each core its x shard TRANSPOSED and cast to bf16
    (xT [400, BC]) — a pure layout/dtype choice, so feature-major tiles are
    directly DMA-able and no PE transposes are needed. (Accuracy is identical
    to v1, which also cast x to bf16 on-chip before use.)
  - The per-AE feature gather x[:, idx] happens ON DEVICE via SWDGE
    dma_gather: 400 feature rows of xT are gathered into AE-contiguous
    ("gathered") order, 8KB contiguous per descriptor, into SBUF supertiles
    xg [128, 4, SUP] (partition = gathered feature, free = sample).
  - In gathered order every matmul is block-diagonal: encode, decode and the
    per-AE err^2 group-sum each need only 4 PE passes per 512-sample tile
    (vs 12/12/4 dense + 16 transposes in v1): chunk c<3 = AEs 8c..8c+7
    (128 feats -> 96 hidden), chunk 3 = AE 24 (16 feats -> 12 hidden).
  - PSUM: enc pool 3 banks (AE24's 12 hidden packed into bank 0 rows 96:108
    via tile_position so the encode sigmoid is ONE ACT over 3 banks),
    dec pool 4 banks, G-stripes pool 1 bank — exactly 8.
  - The main loop is software-pipelined 4 stages deep
    (enc(i) | dec(i-1) | err^2(i-2) | G(i-3)) so the scalar<->tensor
    dependency chain of one tile never serializes the engines.
  - sqrt(mean+eps) is phase-split to the end; final sum over the 25 AEs is a
    ones-block fp32 matmul.
"""

import sys

for _p in ("/opt/trn_rl_repo", "/opt/pypackages"):
    if _p not in sys.path:
        sys.path.append(_p)

import numpy as np

B = 131072
F = 400          # features
N_AE = 25
KF = 16          # features per AE
H = 12           # hidden per AE
EPS = 1e-6
N_CORES = 8
BC = B // N_CORES    # 16384 samples per core
NB = 512             # batch tile (matmul moving free dim)
NT = BC // NB        # 32 tiles per core
SUP = 4096           # samples per dma_gather super-tile
NSUP = BC // SUP
TPS = SUP // NB      # tiles per super
NI = 512             # gather idx slots (400 valid + 112 negative pad)

_NC_CACHE = {}
HOST_GATHER = False   # gather runs on device via SWDGE dma_gather
WARM_DEC = 2          # keep-warm PE passes at the head of each dec stage
WARM_G = 2            # keep-warm PE passes at the head of each G stage
WARM_N = 512          # columns per keep-warm pass


def _build_nc(with_bias: bool, host_gather: bool):
    import concourse.tile as tile
    from concourse import bacc, mybir

    f32 = mybir.dt.float32
    bf16 = mybir.dt.bfloat16
    i16 = mybir.dt.int16
    AF = mybir.ActivationFunctionType

    nc = bacc.Bacc()

    xt_d = nc.declare_dram_parameter("xt", [F, BC], bf16, isOutput=False)
    idx_d = nc.declare_dram_parameter("idxw", [128, NI // 16], i16, isOutput=False)
    wenc_d = nc.declare_dram_parameter("wenc", [4, 128, 96], bf16, isOutput=False)
    wdec_d = nc.declare_dram_parameter("wdec", [4, 128, 128], bf16, isOutput=False)
    g_d = nc.declare_dram_parameter("gmat", [4, 128, 32], bf16, isOutput=False)
    hb_d = nc.declare_dram_parameter("hbm", [128, 3], f32, isOutput=False)
    vb_d = nc.declare_dram_parameter("vbm", [128, 4], f32, isOutput=False)
    y_d = nc.declare_dram_parameter("y", [BC], f32, isOutput=True)

    with tile.TileContext(nc) as tc:
        with (
            tc.tile_pool(name="singles", bufs=1) as singles,
            tc.tile_pool(name="xg", bufs=3) as xg_p,
            tc.tile_pool(name="ht", bufs=2) as ht_p,
            tc.tile_pool(name="rec", bufs=3) as rec_p,
            tc.tile_pool(name="encp", bufs=1, space="PSUM") as encp_p,
            tc.tile_pool(name="decp", bufs=1, space="PSUM") as decp_p,
            tc.tile_pool(name="gp", bufs=1, space="PSUM") as gp_p,
        ):
            # --- constants ---
            ib = singles.tile([128, NI // 16], i16)
            nc.sync.dma_start(out=ib, in_=idx_d[:, :])
            wenc_sb = singles.tile([128, 4, 96], bf16)
            nc.sync.dma_start(
                out=wenc_sb, in_=wenc_d[:, :, :].rearrange("c p n -> p c n")
            )
            wdec_sb = singles.tile([128, 4, 128], bf16)
            nc.sync.dma_start(
                out=wdec_sb, in_=wdec_d[:, :, :].rearrange("c p n -> p c n")
            )
            g_sb = singles.tile([128, 4, 32], bf16)
            nc.sync.dma_start(out=g_sb, in_=g_d[:, :, :].rearrange("c p n -> p c n"))
            if with_bias:
                hb_sb = singles.tile([128, 3], f32)
                nc.sync.dma_start(out=hb_sb, in_=hb_d[:, :])
                vb_sb = singles.tile([128, 4], f32)
                nc.sync.dma_start(out=vb_sb, in_=vb_d[:, :])
            # raw S sums per 4-tile group, then sqrt(S/16+eps) in bf16
            sall = singles.tile([128, NT // 4, NB], f32)
            srec = singles.tile([128, NT // 4, NB], bf16)
            ybuf = singles.tile([4, NT // 4, NB], f32)
            eps_sb = singles.tile([128, 1], f32)
            nc.vector.memset(eps_sb, EPS)
            ones4 = singles.tile([128, 4], bf16)
            nc.gpsimd.memset(ones4, 0.0)
            for g in range(4):
                nc.gpsimd.memset(ones4[32 * g:32 * g + N_AE, g:g + 1], 1.0)
            # warm the Sqrt activation table during startup dead time
            nc.scalar.activation(
                out=srec[0:1, 0, 0:1], in_=eps_sb[0:1, :], func=AF.Sqrt
            )
            # zero tile for keep-warm PE passes (PE_HAM throttles an idle PE
            # to 1.2 GHz; zero-weight passes into a bank that the next real
            # matmul start=True-resets keep it at 2.4 GHz for free)
            zjunk = singles.tile([128, NB], bf16)
            nc.vector.memset(zjunk, 0.0)

            xgs = [None] * NSUP
            hts = [None] * NT
            recs = [None] * NT
            gps = [None] * (NT // 4)

            def _gather_into(dst, lo, n):
                if host_gather:
                    # xt rows are already in gathered order: plain affine loads
                    nc.sync.dma_start(
                        out=dst[:, 0:3, :],
                        in_=xt_d[0:384, lo:lo + n].rearrange(
                            "(c p) s -> p c s", p=128
                        ),
                    )
                    nc.sync.dma_start(
                        out=dst[0:16, 3, :], in_=xt_d[384:400, lo:lo + n]
                    )
                    return
                nc.gpsimd.dma_gather(
                    out_ap=dst[:, :, :],
                    in_ap=xt_d[:, lo:lo + n],
                    idxs_ap=ib[:, :],
                    num_idxs=NI,
                    num_idxs_reg=F,
                    elem_size=n,
                    elem_step=BC,
                )

            # super 0 is split [1024, 3072] into persistent tiles so compute
            # starts after a short first gather instead of a full 4096 one
            xq0 = singles.tile([128, 4, 2 * NB], bf16)
            xq1 = singles.tile([128, 4, 6 * NB], bf16)

            def gather(s):
                xgs[s] = xg_p.tile([128, 4, SUP], bf16, tag="xg", name="xg")
                _gather_into(xgs[s], s * SUP, SUP)

            def xg_of(t):
                if t < 2:
                    return xq0[:, :, t * NB:(t + 1) * NB]
                if t < TPS:
                    return xq1[:, :, (t - 2) * NB:(t - 1) * NB]
                return xgs[t // TPS][:, :, (t % TPS) * NB:(t % TPS) * NB + NB]

            def enc(t):
                pe_ = encp_p.tile([128, 3, NB], f32, tag="encp")
                xv = xg_of(t)
                for c in range(3):
                    nc.tensor.matmul(
                        pe_[0:96, c, :],
                        lhsT=wenc_sb[0:128, c, 0:96],
                        rhs=xv[0:128, c, :],
                        start=True,
                        stop=True,
                    )
                nc.tensor.matmul(
                    pe_[96:108, 0, :],
                    lhsT=wenc_sb[0:16, 3, 0:12],
                    rhs=xv[0:16, 3, :],
                    start=True,
                    stop=True,
                    tile_position=(0, 96),
                )
                ht = hts[t] = ht_p.tile([128, 3, NB], bf16, tag="ht", name="ht")
                if with_bias:
                    for c in range(3):
                        nc.scalar.activation(
                            out=ht[0:96, c, :],
                            in_=pe_[0:96, c, :],
                            func=AF.Sigmoid,
                            bias=hb_sb[0:96, c:c + 1],
                        )
                    nc.scalar.activation(
                        out=ht[96:108, 0, :],
                        in_=pe_[96:108, 0, :],
                        func=AF.Sigmoid,
                        bias=hb_sb[96:108, 0:1],
                    )
                else:
                    nc.scalar.activation(
                        out=ht[0:128, :, :], in_=pe_[0:128, :, :], func=AF.Sigmoid
                    )

            def dec(t, warm=WARM_DEC):
                ht = hts[t]
                hts[t] = None
                pd = decp_p.tile([128, 4, NB], f32, tag="decp")
                for k in range(warm):
                    nc.tensor.matmul(
                        pd[0:128, k % 3, 0:WARM_N],
                        lhsT=zjunk[0:128, 0:128],
                        rhs=zjunk[:, 0:WARM_N],
                        start=True,
                        stop=True,
                    )
                for c in range(3):
                    nc.tensor.matmul(
                        pd[0:128, c, :],
                        lhsT=wdec_sb[0:128, c, 0:128],
                        rhs=ht[0:128, c, :],
                        start=True,
                        stop=True,
                    )
                nc.tensor.matmul(
                    pd[0:16, 3, :],
                    lhsT=wdec_sb[96:108, 3, 0:16],
                    rhs=ht[96:108, 0, :],
                    start=True,
                    stop=True,
                    tile_position=(96, 0),
                )
                rec = recs[t] = rec_p.tile([128, 4, NB], bf16, tag="rec", name="rec")
                if with_bias:
                    for c in range(4):
                        nc.scalar.activation(
                            out=rec[0:128, c, :],
                            in_=pd[0:128, c, :],
                            func=AF.Sigmoid,
                            bias=vb_sb[0:128, c:c + 1],
                        )
                else:
                    nc.scalar.activation(out=rec, in_=pd[:, :, :], func=AF.Sigmoid)

            def errsq(t):
                rec = recs[t]
                nc.vector.tensor_sub(rec, xg_of(t), rec)
                nc.vector.tensor_mul(rec, rec, rec)

            def sqrt_stage(j):
                nc.scalar.activation(
                    out=srec[:, j, :],
                    in_=sall[:, j, :],
                    func=AF.Sqrt,
                    bias=eps_sb,
                    scale=1.0 / KF,
                )

            def py_stage(j):
                py = gp_p.tile([128, NB], f32, tag="gp", name="py")
                nc.tensor.matmul(
                    py[0:4, :],
                    lhsT=ones4,
                    rhs=srec[:, j, :],
                    start=True,
                    stop=True,
                )
                nc.vector.tensor_copy(out=ybuf[:, j, :], in_=py[0:4, :])

            def gsum(t, warm=WARM_G):
                rec = recs[t]
                recs[t] = None
                g = t % 4
                if g == 0:
                    gps[t // 4] = gp_p.tile([128, NB], f32, tag="gp", name="gp")
                ps4 = gps[t // 4]
                for k in range(warm):
                    nc.tensor.matmul(
                        ps4[32 * g:32 * (g + 1), 0:WARM_N],
                        lhsT=zjunk[0:128, 0:32],
                        rhs=zjunk[:, 0:WARM_N],
                        start=True,
                        stop=True,
                        tile_position=(0, 32 * g),
                    )
                for c in range(3):
                    nc.tensor.matmul(
                        ps4[32 * g:32 * (g + 1), :],
                        lhsT=g_sb[0:128, c, :],
                        rhs=rec[0:128, c, :],
                        start=(c == 0),
                        stop=False,
                        tile_position=(0, 32 * g),
                    )
                nc.tensor.matmul(
                    ps4[32 * g:32 * (g + 1), :],
                    lhsT=g_sb[0:16, 3, :],
                    rhs=rec[0:16, 3, :],
                    start=False,
                    stop=True,
                    tile_position=(0, 32 * g),
                )
                if g == 3:
                    j = t // 4
                    nc.vector.tensor_copy(out=sall[:, j, :], in_=ps4)
                    gps[j] = None
                    if j >= 1:
                        sqrt_stage(j - 1)
                    if j >= 2:
                        py_stage(j - 2)

            # ---- software-pipelined main loop:
            # step i: enc(i) | dec(i-1) | errsq(i-2) | gsum(i-3)
            _gather_into(xq0, 0, 2 * NB)
            _gather_into(xq1, 2 * NB, 6 * NB)
            wpd = decp_p.tile([128, 4, NB], f32, tag="decp")
            for k in range(56):
                nc.tensor.matmul(
                    wpd[0:128, k % 4, :],
                    lhsT=zjunk[0:128, 0:128],
                    rhs=zjunk[:, :],
                    start=True,
                    stop=True,
                )
            for i in range(NT + 3):
                if 0 <= i - 3 < NT:
                    gsum(i - 3)
                if 0 <= i - 1 < NT:
                    dec(i - 1)
                if i < NT:
                    if i % TPS == 0 and i // TPS + 1 < NSUP:
                        gather(i // TPS + 1)
                    enc(i)
                if 0 <= i - 2 < NT:
                    errsq(i - 2)

            sqrt_stage(NT // 4 - 1)
            py_stage(NT // 4 - 2)
            py_stage(NT // 4 - 1)

            # y[b], b = t*NB + i, t = 4j + g  ->  y view [g, j, i]
            y_ap = y_d[:].rearrange("(j g i) -> g j i", g=4, i=NB)
            nc.sync.dma_start(out=y_ap, in_=ybuf)

    nc.compile()
    return nc


def _host_prep(W, hb, vb, idx):
    import ml_dtypes

    bf16 = ml_dtypes.bfloat16
    W = np.asarray(W, np.float32)
    hb = np.asarray(hb, np.float32)
    vb = np.asarray(vb, np.float32)
    idx = np.asarray(idx)

    # gathered order: position i = 128*c + 16*a_local + k
    chunk_aes = [list(range(8 * c, 8 * c + 8)) for c in range(3)] + [[24]]

    idxs = np.full((NI,), -1, np.int16)
    wenc = np.zeros((4, 128, 96), np.float32)
    wdec = np.zeros((4, 128, 128), np.float32)
    gmat = np.zeros((4, 128, 32), np.float32)
    hbm = np.zeros((128, 3), np.float32)
    vbm = np.zeros((128, 4), np.float32)
    for c, aes in enumerate(chunk_aes):
        for al, a in enumerate(aes):
            for k in range(KF):
                r = al * KF + k
                idxs[128 * c + r] = idx[a, k]
                if c < 3:
                    wenc[c, r, al * H:(al + 1) * H] = W[a, k, :]
                    wdec[c, al * H:(al + 1) * H, r] = W[a, k, :]
                    gmat[c, r, a] = 1.0
                    vbm[r, c] = vb[a, k]
                else:
                    wenc[3, k, 0:H] = W[a, k, :]
                    wdec[3, 96:96 + H, k] = W[a, k, :]
                    gmat[3, k, a] = 1.0
                    vbm[k, 3] = vb[a, k]
            if c < 3:
                hbm[al * H:(al + 1) * H, c] = hb[a, :]
            else:
                hbm[96:96 + H, 0] = hb[a, :]

    idxw = np.zeros((128, NI // 16), np.int16)
    for j in range(NI):
        idxw[j % 16::16, j // 16] = idxs[j]

    order = np.zeros((F,), np.int64)
    for j in range(F):
        order[j] = idxs[j]

    return {
        "_order": order,
        "idxw": idxw,
        "wenc": wenc.astype(bf16),
        "wdec": wdec.astype(bf16),
        "gmat": gmat.astype(bf16),
        "hbm": hbm,
        "vbm": vbm,
    }


def _get_nc(with_bias):
    key = ("nc", with_bias, HOST_GATHER, WARM_DEC, WARM_G, WARM_N)
    if key not in _NC_CACHE:
        _NC_CACHE[key] = _build_nc(with_bias, HOST_GATHER)
    return _NC_CACHE[key]


def _run(x, W, hb, vb, idx, trace=False):
    import ml_dtypes
    from concourse.bass_utils import run_bass_kernel_spmd

    bf16 = ml_dtypes.bfloat16
    consts = _host_prep(W=W, hb=hb, vb=vb, idx=idx)
    with_bias = bool(np.any(np.asarray(hb)) or np.any(np.asarray(vb)))
    x = np.asarray(x, np.float32)
    if HOST_GATHER:
        order = np.asarray(consts.pop("_order"))
    else:
        consts.pop("_order")
    in_maps = []
    for c in range(N_CORES):
        xt = np.ascontiguousarray(x[c * BC:(c + 1) * BC].T).astype(bf16)
        if HOST_GATHER:
            xt = np.ascontiguousarray(xt[order])
        in_maps.append({"xt": xt, **consts})
    nc = _get_nc(with_bias)
    res = run_bass_kernel_spmd(nc, in_maps, list(range(N_CORES)), trace=trace)
    y = np.concatenate([res.results[c]["y"] for c in range(N_CORES)])
    return y, res


def kernel(x, W, hb, vb, idx):
    y, _ = _run(x, W, hb, vb, idx)
    return y
